# revision 48
# baseline (speedup 1.0000x reference)
"""Trainium2 Bass kernel for nn_AttentionLayer (B=4, L=S=2048, D=1024, H=16).

Sharding: 8 cores = (batch b in 0..3) x (head-group g in 0..1); each core
handles one batch and 8 heads (512 of the 1024 q/k/v/o channels).

Per-core kernel (bf16 matmuls, fp32 PSUM), fully software-pipelined in
512-wide l/s quarters so projections, attention, out-projection and DMA all
overlap:

  - q^T/k^T are kept packed: e-block m holds heads (2m, 2m+1) on partition
    halves [0:64] / [64:128].  Scores for the two heads of a pair are issued
    as 64x128 PE row-tiles (tile_position (0,0) / (64,0), contraction E=64),
    which avoids any zero-padded per-head k layout (no memsets, single
    k-proj copy).
  - exp (ScalarE) is evaluated once per s-block over BOTH heads' score
    planes ([128, 2, W] access pattern) to halve activation call overhead.
  - causal masking multiplies the 128-wide diagonal windows of the exp
    output by a lower-block-triangular bf16 tile (VectorE), only for the
    diagonal group of each chunk.
  - AV: O^T = V_ext^T @ P^T with V_ext = [V | 1 | 0-pad to 128 cols, which
    makes LDWEIGHTS FWL-eligible]; PSUM row 64 accumulates the softmax
    denominator.  Divide: PSUM row -> SBUF -> DMA shift to partition 0 ->
    GpSimd partition-broadcast -> DVE reciprocal+multiply.
  - out-projection per l-chunk (delayed one chunk) so its matmuls and the
    fp-> bf16 output DMA overlap the next chunk's attention.
  - (bk is dropped: softmax is invariant to a per-key constant shift; bv
    and bo are added on the host: out += Wo @ bv + bo exactly.)

Host: shards/transposes/casts inputs, runs the SPMD kernel on 8 cores,
sums the two head-group partial outputs per batch, adds Wo@bv + bo.
"""

import numpy as np
import ml_dtypes

B, L, S, D, H, E = 4, 2048, 2048, 1024, 16, 64
NCORES = 8
GROUPS = 2                 # head-groups (tensor-parallel dimension)
HC = H // GROUPS           # heads per core = 8
EC = HC * E                # channels per core = 512
CH = 512                   # attention l-chunk / projection quarter size

_BF16 = ml_dtypes.bfloat16


def build(debug=False):
    import concourse.bass as bass
    import concourse.mybir as mybir
    import concourse.tile as tile
    from concourse import bacc
    from concourse.masks import make_upper_triangular

    f32 = mybir.dt.float32
    bf16 = mybir.dt.bfloat16

    KD = D // 128           # k-tiles over d = 8
    MB = EC // 128          # e-blocks (= head pairs) per core = 4
    NQ = L // CH            # quarters / chunks = 4
    SBC = CH // 128         # s-blocks per chunk = 4
    NSB = S // 128          # total s-blocks = 16
    DC = 512                # out-proj d tile
    NDC = D // DC           # = 2

    nc = bacc.Bacc(None, target_bir_lowering=False, debug=debug)

    qT = nc.dram_tensor("qT", [D, L], bf16, kind="ExternalInput")
    kT = nc.dram_tensor("kT", [D, S], bf16, kind="ExternalInput")
    vT = nc.dram_tensor("vT", [D, S], bf16, kind="ExternalInput")
    wq = nc.dram_tensor("wq", [D, EC], bf16, kind="ExternalInput")
    wk = nc.dram_tensor("wk", [D, EC], bf16, kind="ExternalInput")
    wv = nc.dram_tensor("wv", [D, EC], bf16, kind="ExternalInput")
    wo = nc.dram_tensor("wo", [EC, D], bf16, kind="ExternalInput")
    bq = nc.dram_tensor("bq", [128, MB], f32, kind="ExternalInput")
    out = nc.dram_tensor("out", [L, D], bf16, kind="ExternalOutput")

    scale = 1.0 / float(np.sqrt(E))
    Exp = mybir.ActivationFunctionType.Exp

    with tile.TileContext(nc) as tc:
        with (
            tc.tile_pool(name="persist", bufs=1) as pp,
            tc.tile_pool(name="weights", bufs=1) as wp,
            tc.tile_pool(name="inputs", bufs=4) as ip,
            tc.tile_pool(name="pexp", bufs=3) as p4p,
            tc.tile_pool(name="divp", bufs=2) as dp,
            tc.tile_pool(name="oout", bufs=3) as op_,
            tc.tile_pool(name="psS", bufs=2, space="PSUM") as psS,
            tc.tile_pool(name="psO", bufs=2, space="PSUM") as psO,
        ):
            # ---- persistent SBUF tensors ----
            q_sb = pp.tile([128, MB, L], bf16, tag="q_sb")
            k_sb = pp.tile([128, MB, S], bf16, tag="k_sb")
            v_sb = pp.tile([128, NSB, HC, 128], bf16, tag="v_sb")
            o_sb = pp.tile([128, MB, L], bf16, tag="o_sb")
            tri2 = pp.tile([128, 2, 128], bf16, tag="tri2")
            bq_t = pp.tile([128, MB], f32, tag="bq_t")

            # cols E..127 of each per-head V stationary: a ones column at
            # E (softmax denominator accumulates in PSUM partition E) then
            # zero padding to 128 columns so LDWEIGHTS takes the FWL path
            # (emitted first: runs on idle DVE during the initial DMA wait)
            nc.vector.memset(v_sb[:, :, :, E + 1 :], 0.0)
            nc.vector.memset(v_sb[:, :, :, E : E + 1], 1.0)
            for hh in range(2):
                make_upper_triangular(nc, tri2[:, hh, :], val=1.0, diag=True)
            nc.sync.dma_start(bq_t[:], bq[:])

            wq_sb = wp.tile([128, KD, EC], bf16, tag="wq")
            wk_sb = wp.tile([128, KD, EC], bf16, tag="wk")
            wv_sb = wp.tile([128, KD, EC], bf16, tag="wv")
            wo_sb = wp.tile([128, MB, D], bf16, tag="wo")

            # Weights: big DMAs (341 GB/s vs ~138 at 128 KiB) on the ACT
            # HWDGE ring so they don't queue behind input slabs; first wv
            # piece lands in ~1.7 us so the first v-proj matmul starts early.
            for w_sb, w_dram in ((wv_sb, wv), (wq_sb, wq), (wk_sb, wk)):
                for s in range(2):
                    k0 = 4 * s
                    nc.scalar.dma_start(
                        w_sb[:, k0 : k0 + 4, :],
                        w_dram[128 * k0 : 128 * (k0 + 4), :].rearrange(
                            "(k p) c -> p k c", p=128
                        ),
                    )
            nc.scalar.dma_start(
                wo_sb[:], wo[:, :].rearrange("(m p) c -> p m c", p=128)
            )

            vt, qt, kt = {}, {}, {}

            def _ld(dram, d, n, nsplit=1):
                # one 1 MiB slab per (tensor, quarter): [128, KD, CH].
                # nsplit>1 lands the slab in k-tile pieces so quarter-0
                # consumers can start before the whole slab arrives.
                tq = ip.tile([128, KD, CH], bf16, tag="inq",
                             name=f"in_{dram.name}_{n}")
                kstep = KD // nsplit
                for s in range(nsplit):
                    k0 = s * kstep
                    nc.sync.dma_start(
                        tq[:, k0 : k0 + kstep, :],
                        dram[128 * k0 : 128 * (k0 + kstep),
                             CH * n : CH * (n + 1)].rearrange(
                            "(k p) c -> p k c", p=128
                        ),
                    )
                d[n] = tq

            # quarter-0 slabs, consumption order, in k-tile pieces so the
            # first projection matmuls start after ~256 KiB, not 1 MiB
            _ld(vT, vt, 0, nsplit=4)
            _ld(qT, qt, 0, nsplit=2)
            _ld(kT, kt, 0, nsplit=2)


            # ---- projection building blocks ----
            def vproj_block(sb, n):
                ps = psS.tile([128, EC], f32, tag="sps", name=f"vp{sb}")
                qq = sb % SBC
                for k in range(KD):
                    nc.tensor.matmul(
                        ps[:, :],
                        vt[n][:, k, 128 * qq : 128 * (qq + 1)],
                        wv_sb[:, k, :],
                        start=(k == 0),
                        stop=(k == KD - 1),
                    )
                nc.vector.tensor_copy(
                    v_sb[:, sb, :, 0:E],
                    ps[:, :].rearrange("p (h e) -> p h e", h=HC),
                )

            def qproj_block(m, n):
                ps = psS.tile([128, CH], f32, tag="sps", name=f"qp{m}_{n}")
                for k in range(KD):
                    nc.tensor.matmul(
                        ps[:, :],
                        wq_sb[:, k, 128 * m : 128 * (m + 1)],
                        qt[n][:, k, :],
                        start=(k == 0),
                        stop=(k == KD - 1),
                    )
                nc.vector.tensor_scalar_add(
                    q_sb[:, m, CH * n : CH * (n + 1)], ps[:, :], bq_t[:, m : m + 1]
                )

            def kproj_block(m, n):
                ps = psS.tile([128, CH], f32, tag="sps", name=f"kp{m}_{n}")
                for k in range(KD):
                    nc.tensor.matmul(
                        ps[:, :],
                        wk_sb[:, k, 128 * m : 128 * (m + 1)],
                        kt[n][:, k, :],
                        start=(k == 0),
                        stop=(k == KD - 1),
                    )
                nc.vector.tensor_copy(k_sb[:, m, CH * n : CH * (n + 1)], ps[:, :])

            def oproj_block(lb):
                # psO pool: avoids contending with scores for "sps" slots.
                # Both 512-wide d-groups evacuate into one tile -> a single
                # 256 KiB row DMA (better rate, half the SP-queue slots).
                ot = op_.tile([128, NDC, DC], bf16, tag="ot", name=f"ot{lb}")
                for dc in range(NDC):
                    pso = psO.tile([128, DC], f32, tag="ops",
                                   name=f"op{lb}_{dc}")
                    for mm in range(MB):
                        nc.tensor.matmul(
                            pso[:, :],
                            o_sb[:, mm, 128 * lb : 128 * (lb + 1)],
                            wo_sb[:, mm, DC * dc : DC * (dc + 1)],
                            start=(mm == 0),
                            stop=(mm == MB - 1),
                        )
                    nc.vector.tensor_copy(ot[:, dc, :], pso[:, :])
                nc.sync.dma_start(
                    out[128 * lb : 128 * (lb + 1), :], ot[:, :, :]
                )

            # ---- attention: one chunk, all 4 head pairs as a single
            # software-pipelined stream (scores/exp lead AV by LAG blocks,
            # continuing straight across pair boundaries so the exp backlog
            # never stalls the tensor engine). 2-block batches halve the PE
            # tiling-mode switches.
            def scores_block(st, m, i, n):
                nsb = SBC * (n + 1)
                g, ii = divmod(i, SBC)
                diag = g == n
                c0 = 128 * ii if diag else 0
                if ii == 0:
                    st[m, "p4", g] = p4p.tile(
                        [128, 2, SBC, CH], bf16, tag="p4",
                        name=f"p4_{m}_{n}_{g}")
                p4 = st[m, "p4", g]
                sps = psS.tile([128, 2, CH], f32, tag="sps",
                               name=f"sps{m}_{n}_{i}")
                nc.tensor.matmul(
                    sps[:, 0, c0:CH],
                    k_sb[0:64, m, 128 * i : 128 * (i + 1)],
                    q_sb[0:64, m, CH * n + c0 : CH * (n + 1)],
                    start=True, stop=True,
                    tile_position=(0, 0),
                )
                nc.tensor.matmul(
                    sps[:, 1, c0:CH],
                    k_sb[64:128, m, 128 * i : 128 * (i + 1)],
                    q_sb[64:128, m, CH * n + c0 : CH * (n + 1)],
                    start=True, stop=True,
                    tile_position=(64, 0),
                )
                nc.scalar.activation(
                    p4[:, :, ii, c0:CH], sps[:, :, c0:CH], Exp, scale=scale,
                )
                if diag:  # causal mask on the 128-wide diagonal window
                    nc.vector.tensor_mul(
                        p4[:, :, ii, c0 : c0 + 128],
                        p4[:, :, ii, c0 : c0 + 128],
                        tri2[:, :, :],
                    )

            def av_block(st, m, i, n):
                nsb = SBC * (n + 1)
                g, ii = divmod(i, SBC)
                diag = g == n
                c0 = 128 * ii if diag else 0
                if i == 0:
                    st[m, "ops"] = psO.tile([128, 2, CH], f32, tag="ops",
                                            name=f"ops{m}_{n}")
                ops = st[m, "ops"]
                p4 = st[m, "p4", g]
                for hh in range(2):
                    nc.tensor.matmul(
                        ops[:, hh, c0:CH],
                        v_sb[:, i, 2 * m + hh, :],
                        p4[:, hh, ii, c0:CH],
                        start=(i == 0),
                        stop=(i == nsb - 1),
                    )

            def divide_chain(st, m, n):
                # softmax divide: denominators live in PSUM partition E
                ops = st[m, "ops"]
                rs = dp.tile([128, 2, CH], f32, tag="rs", name=f"rs{m}_{n}")
                nc.vector.tensor_copy(rs[E : E + 1, :, :], ops[E : E + 1, :, :])
                dn = dp.tile([128, 2 * CH], f32, tag="dn", name=f"dn{m}_{n}")
                nc.sync.dma_start(dn[0:1, :], rs[E : E + 1, :, :])
                rr = dp.tile([128, 2, CH], f32, tag="rr", name=f"rr{m}_{n}")
                nc.gpsimd.partition_broadcast(rr[0:E, :, :], dn[0:1, :],
                                              channels=E)
                nc.vector.reciprocal_approx_fast(rr[0:E, :, :], rr[0:E, :, :])
                o_tmp = dp.tile([128, 2, CH], bf16, tag="o_tmp",
                                name=f"otmp{m}_{n}")
                nc.vector.tensor_mul(
                    o_tmp[0:E, :, :], ops[0:E, :, :], rr[0:E, :, :]
                )
                nc.sync.dma_start(
                    o_sb[0:E, m, CH * n : CH * (n + 1)], o_tmp[0:E, 0, :]
                )
                nc.sync.dma_start(
                    o_sb[E:128, m, CH * n : CH * (n + 1)], o_tmp[0:E, 1, :]
                )

            # ---- quarter 0 projections (straight-line) ----
            for sb in range(SBC):
                vproj_block(sb, 0)
            # dependency-free fill while the q/k input slabs stream in:
            # keeps the PE busy (HAM at 8/8) through the quarter-0 DMA wait
            psw0 = psS.tile([128, 256], f32, tag="sps", name="warm0")
            for r in range(10):
                nc.tensor.matmul(
                    psw0[:, :], tri2[:, 0, :], tri2[:, :, :],
                    start=(r == 0), stop=(r == 9),
                )
            for m in range(MB):
                qproj_block(m, 0)
            for m in range(MB):
                kproj_block(m, 0)

            # ---- chunks: attention n overlapped with projections of
            #      quarter n+1 and out-projection of chunk n-1 ----
            for n in range(NQ):
                filler = []
                if n >= 1:  # out-proj of the previous chunk: deps all met
                    for lb in range(SBC * (n - 1), SBC * n):
                        filler.append((oproj_block, (lb,)))
                if n + 1 < NQ:
                    for sb in range(SBC * (n + 1), SBC * (n + 2)):
                        filler.append((vproj_block, (sb, n + 1)))
                    for m in range(MB):
                        filler.append((qproj_block, (m, n + 1)))
                    for m in range(MB):
                        filler.append((kproj_block, (m, n + 1)))

                if n + 1 < NQ:
                    _ld(vT, vt, n + 1)
                nsb = SBC * (n + 1)
                blocks = [(m, i) for m in range(MB) for i in range(nsb)]
                LAG = 2
                st = {}
                popped = 0
                av_done = 0
                for idx in range(0, len(blocks) + LAG, 2):
                    if idx < len(blocks):
                        scores_block(st, *blocks[idx], n)
                        scores_block(st, *blocks[idx + 1], n)
                    for j in (idx - LAG, idx - LAG + 1):
                        if j < 0 or j >= len(blocks):
                            continue
                        m, i = blocks[j]
                        av_block(st, m, i, n)
                        av_done += 1
                        if i == nsb - 1:  # pair finished
                            divide_chain(st, m, n)
                            if m == 0 and n + 1 < NQ:
                                _ld(qT, qt, n + 1)
                            if m == 1 and n + 1 < NQ:
                                _ld(kT, kt, n + 1)
                    # fine-grained filler pops: spread across the whole
                    # chunk so each filler's PSUM-slot wait overlaps work
                    want = (len(filler) * av_done) // len(blocks)
                    while popped < want:
                        fn, args = filler[popped]
                        fn(*args)
                        popped += 1


            # Keep the PE busy (and the HAM clock-gate at 8/8) while the
            # final pair's softmax-divide chain drains: dependency-free
            # dummy matmuls on resident data.  Without these the ~6 us stall
            # re-throttles the PE to 1.2 GHz for the whole tail.
            psw = psS.tile([128, 512], f32, tag="sps", name="warm")
            for r in range(24):
                nc.tensor.matmul(
                    psw[:, :], k_sb[:, 0, 0:128], q_sb[:, 0, 0:512],
                    start=(r == 0), stop=(r == 23),
                )

            # tail: out-projection of the last chunk; lb 12-13's PSUM
            # groups open with e-blocks 0..2 before the final divide chain
            # lands, e-block 3 closes them after; merged row DMAs.
            open4 = {}
            for lb in (12, 13):
                for dc in range(NDC):
                    g = 2 * (lb - 12) + dc
                    pool, tag = ((psO, "ops") if g < 2 else (psS, "sps"))
                    pso = pool.tile([128, DC], f32, tag=tag,
                                    name=f"opt{lb}_{dc}")
                    for mm in range(MB - 1):
                        nc.tensor.matmul(
                            pso[:, :],
                            o_sb[:, mm, 128 * lb : 128 * (lb + 1)],
                            wo_sb[:, mm, DC * dc : DC * (dc + 1)],
                            start=(mm == 0),
                            stop=False,
                        )
                    open4[lb, dc] = pso
            for lb in (12, 13):
                ot = op_.tile([128, NDC, DC], bf16, tag="ot", name=f"ott{lb}")
                for dc in range(NDC):
                    pso = open4[lb, dc]
                    nc.tensor.matmul(
                        pso[:, :],
                        o_sb[:, MB - 1, 128 * lb : 128 * (lb + 1)],
                        wo_sb[:, MB - 1, DC * dc : DC * (dc + 1)],
                        start=False,
                        stop=True,
                    )
                    nc.vector.tensor_copy(ot[:, dc, :], pso[:, :])
                nc.sync.dma_start(
                    out[128 * lb : 128 * (lb + 1), :], ot[:, :, :]
                )
            for lb in (14, 15):
                oproj_block(lb)

    nc.compile()
    return nc


def _prep_inputs(queries, keys, values, Wq, bq, Wk, Wv, Wo):
    """Build the 8 per-core input maps (host-side shard + transpose + cast)."""
    MB = EC // 128
    in_maps = []
    qT = [np.ascontiguousarray(queries[b].T.astype(_BF16)) for b in range(B)]
    kT = [np.ascontiguousarray(keys[b].T.astype(_BF16)) for b in range(B)]
    vT = [np.ascontiguousarray(values[b].T.astype(_BF16)) for b in range(B)]
    wqs, wks, wvs, wos, bqs = [], [], [], [], []
    for g in range(GROUPS):
        sl = slice(g * EC, (g + 1) * EC)
        wqs.append(np.ascontiguousarray(Wq[sl, :].T.astype(_BF16)))
        wks.append(np.ascontiguousarray(Wk[sl, :].T.astype(_BF16)))
        wvs.append(np.ascontiguousarray(Wv[sl, :].T.astype(_BF16)))
        wos.append(np.ascontiguousarray(Wo[:, sl].T.astype(_BF16)))
        bqs.append(
            np.ascontiguousarray(
                bq[sl].astype(np.float32).reshape(MB, 128).T
            )
        )
    for c in range(NCORES):
        b, g = c // GROUPS, c % GROUPS
        in_maps.append(
            {
                "qT": qT[b], "kT": kT[b], "vT": vT[b],
                "wq": wqs[g], "wk": wks[g], "wv": wvs[g],
                "wo": wos[g], "bq": bqs[g],
            }
        )
    return in_maps


_NC_CACHE = {}


def kernel(queries, keys, values, attn_mask, Wq, bq, Wk, bk, Wv, bv, Wo, bo,
           _trace=False):
    from concourse.bass_utils import run_bass_kernel_spmd

    queries = np.asarray(queries, np.float32)
    keys = np.asarray(keys, np.float32)
    values = np.asarray(values, np.float32)
    Wq, Wk, Wv, Wo = (np.asarray(a, np.float32) for a in (Wq, Wk, Wv, Wo))
    bq, bk, bv, bo = (np.asarray(a, np.float32) for a in (bq, bk, bv, bo))

    if "nc" not in _NC_CACHE:
        _NC_CACHE["nc"] = build()
    nc = _NC_CACHE["nc"]

    in_maps = _prep_inputs(queries, keys, values, Wq, bq, Wk, Wv, Wo)
    res = run_bass_kernel_spmd(
        nc, in_maps, core_ids=list(range(NCORES)), trace=_trace
    )
    _NC_CACHE["last_results"] = res

    out = np.zeros((B, L, D), np.float32)
    for c in range(NCORES):
        out[c // GROUPS] += res.results[c]["out"].astype(np.float32)
    # bv exits through the (row-sum-1) softmax as Wo @ bv; bo is direct.
    out += (Wo @ bv + bo)[None, None, :]
    return out


# revision 49
# speedup vs baseline: 1.0382x; 1.0382x over previous
"""Trainium2 Bass kernel for nn_AttentionLayer (B=4, L=S=2048, D=1024, H=16).

Sharding: 8 cores = (batch b in 0..3) x (head-group g in 0..1); each core
handles one batch and 8 heads (512 of the 1024 q/k/v/o channels).

Per-core kernel (bf16 matmuls, fp32 PSUM), fully software-pipelined in
512-wide l/s quarters so projections, attention, out-projection and DMA all
overlap:

  - q^T/k^T are kept packed: e-block m holds heads (2m, 2m+1) on partition
    halves [0:64] / [64:128].  Scores for the two heads of a pair are issued
    as 64x128 PE row-tiles (tile_position (0,0) / (64,0), contraction E=64),
    which avoids any zero-padded per-head k layout (no memsets, single
    k-proj copy).
  - exp (ScalarE) is evaluated once per s-block over BOTH heads' score
    planes ([128, 2, W] access pattern) to halve activation call overhead.
  - causal masking multiplies the 128-wide diagonal windows of the exp
    output by a lower-block-triangular bf16 tile (VectorE), only for the
    diagonal group of each chunk.
  - AV: O^T = V_ext^T @ P^T with V_ext = [V | 1 | 0-pad to 128 cols, which
    makes LDWEIGHTS FWL-eligible]; PSUM row 64 accumulates the softmax
    denominator.  Divide: PSUM row -> SBUF -> DMA shift to partition 0 ->
    GpSimd partition-broadcast -> DVE reciprocal+multiply.
  - out-projection per l-chunk (delayed one chunk) so its matmuls and the
    fp-> bf16 output DMA overlap the next chunk's attention.
  - (bk is dropped: softmax is invariant to a per-key constant shift; bv
    and bo are added on the host: out += Wo @ bv + bo exactly.)

Host: shards/transposes/casts inputs, runs the SPMD kernel on 8 cores,
sums the two head-group partial outputs per batch, adds Wo@bv + bo.
"""

import numpy as np
import ml_dtypes

B, L, S, D, H, E = 4, 2048, 2048, 1024, 16, 64
NCORES = 8
GROUPS = 2                 # head-groups (tensor-parallel dimension)
HC = H // GROUPS           # heads per core = 8
EC = HC * E                # channels per core = 512
CH = 512                   # attention l-chunk / projection quarter size

_BF16 = ml_dtypes.bfloat16


def build(debug=False):
    import concourse.bass as bass
    import concourse.mybir as mybir
    import concourse.tile as tile
    from concourse import bacc
    from concourse.masks import make_upper_triangular

    f32 = mybir.dt.float32
    bf16 = mybir.dt.bfloat16

    KD = D // 128           # k-tiles over d = 8
    MB = EC // 128          # e-blocks (= head pairs) per core = 4
    NQ = L // CH            # quarters / chunks = 4
    SBC = CH // 128         # s-blocks per chunk = 4
    NSB = S // 128          # total s-blocks = 16
    DC = 512                # out-proj d tile
    NDC = D // DC           # = 2

    nc = bacc.Bacc(None, target_bir_lowering=False, debug=debug)

    qT = nc.dram_tensor("qT", [D, L], bf16, kind="ExternalInput")
    kT = nc.dram_tensor("kT", [D, S], bf16, kind="ExternalInput")
    vT = nc.dram_tensor("vT", [D, S], bf16, kind="ExternalInput")
    wq = nc.dram_tensor("wq", [D, EC], bf16, kind="ExternalInput")
    wk = nc.dram_tensor("wk", [D, EC], bf16, kind="ExternalInput")
    wv = nc.dram_tensor("wv", [D, EC], bf16, kind="ExternalInput")
    wo = nc.dram_tensor("wo", [EC, D], bf16, kind="ExternalInput")
    bq = nc.dram_tensor("bq", [128, MB], f32, kind="ExternalInput")
    out = nc.dram_tensor("out", [L, D], bf16, kind="ExternalOutput")

    scale = 1.0 / float(np.sqrt(E))
    Exp = mybir.ActivationFunctionType.Exp

    with tile.TileContext(nc) as tc:
        with (
            tc.tile_pool(name="persist", bufs=1) as pp,
            tc.tile_pool(name="weights", bufs=1) as wp,
            tc.tile_pool(name="inputs", bufs=4) as ip,
            tc.tile_pool(name="pexp", bufs=3) as p4p,
            tc.tile_pool(name="divp", bufs=2) as dp,
            tc.tile_pool(name="oout", bufs=3) as op_,
            tc.tile_pool(name="psS", bufs=2, space="PSUM") as psS,
            tc.tile_pool(name="psO", bufs=2, space="PSUM") as psO,
        ):
            # ---- persistent SBUF tensors ----
            q_sb = pp.tile([128, MB, L], bf16, tag="q_sb")
            k_sb = pp.tile([128, MB, S], bf16, tag="k_sb")
            v_sb = pp.tile([128, NSB, HC, 128], bf16, tag="v_sb")
            o_sb = pp.tile([128, MB, L], bf16, tag="o_sb")
            tri2 = pp.tile([128, 2, 128], bf16, tag="tri2")
            bq_t = pp.tile([128, MB], f32, tag="bq_t")

            # cols E..127 of each per-head V stationary: a ones column at
            # E (softmax denominator accumulates in PSUM partition E) then
            # zero padding to 128 columns so LDWEIGHTS takes the FWL path
            # (emitted first: runs on idle DVE during the initial DMA wait)
            nc.vector.memset(v_sb[:, :, :, E + 1 :], 0.0)
            nc.vector.memset(v_sb[:, :, :, E : E + 1], 1.0)
            for hh in range(2):
                make_upper_triangular(nc, tri2[:, hh, :], val=1.0, diag=True)
            nc.sync.dma_start(bq_t[:], bq[:])

            wq_sb = wp.tile([128, KD, EC], bf16, tag="wq")
            wk_sb = wp.tile([128, KD, EC], bf16, tag="wk")
            wv_sb = wp.tile([128, KD, EC], bf16, tag="wv")
            wo_sb = wp.tile([128, MB, D], bf16, tag="wo")

            # Weights: big DMAs (341 GB/s vs ~138 at 128 KiB) on the ACT
            # HWDGE ring so they don't queue behind input slabs; first wv
            # piece lands in ~1.7 us so the first v-proj matmul starts early.
            for w_sb, w_dram in ((wv_sb, wv), (wq_sb, wq), (wk_sb, wk)):
                for s in range(2):
                    k0 = 4 * s
                    nc.scalar.dma_start(
                        w_sb[:, k0 : k0 + 4, :],
                        w_dram[128 * k0 : 128 * (k0 + 4), :].rearrange(
                            "(k p) c -> p k c", p=128
                        ),
                    )
            nc.scalar.dma_start(
                wo_sb[:], wo[:, :].rearrange("(m p) c -> p m c", p=128)
            )

            vt, qt, kt = {}, {}, {}

            def _ld(dram, d, n, nsplit=1):
                # one 1 MiB slab per (tensor, quarter): [128, KD, CH].
                # nsplit>1 lands the slab in k-tile pieces so quarter-0
                # consumers can start before the whole slab arrives.
                tq = ip.tile([128, KD, CH], bf16, tag="inq",
                             name=f"in_{dram.name}_{n}")
                kstep = KD // nsplit
                for s in range(nsplit):
                    k0 = s * kstep
                    nc.sync.dma_start(
                        tq[:, k0 : k0 + kstep, :],
                        dram[128 * k0 : 128 * (k0 + kstep),
                             CH * n : CH * (n + 1)].rearrange(
                            "(k p) c -> p k c", p=128
                        ),
                    )
                d[n] = tq

            # quarter-0 slabs, consumption order, in k-tile pieces so the
            # first projection matmuls start after ~256 KiB, not 1 MiB
            _ld(vT, vt, 0, nsplit=4)
            _ld(qT, qt, 0, nsplit=2)
            _ld(kT, kt, 0, nsplit=2)


            # ---- projection building blocks ----
            def vproj_block(sb, n):
                ps = psS.tile([128, EC], f32, tag="sps", name=f"vp{sb}")
                qq = sb % SBC
                for k in range(KD):
                    nc.tensor.matmul(
                        ps[:, :],
                        vt[n][:, k, 128 * qq : 128 * (qq + 1)],
                        wv_sb[:, k, :],
                        start=(k == 0),
                        stop=(k == KD - 1),
                    )
                nc.vector.tensor_copy(
                    v_sb[:, sb, :, 0:E],
                    ps[:, :].rearrange("p (h e) -> p h e", h=HC),
                )

            def qproj_block(m, n):
                ps = psS.tile([128, CH], f32, tag="sps", name=f"qp{m}_{n}")
                for k in range(KD):
                    nc.tensor.matmul(
                        ps[:, :],
                        wq_sb[:, k, 128 * m : 128 * (m + 1)],
                        qt[n][:, k, :],
                        start=(k == 0),
                        stop=(k == KD - 1),
                    )
                nc.vector.tensor_scalar_add(
                    q_sb[:, m, CH * n : CH * (n + 1)], ps[:, :], bq_t[:, m : m + 1]
                )

            def kproj_block(m, n):
                ps = psS.tile([128, CH], f32, tag="sps", name=f"kp{m}_{n}")
                for k in range(KD):
                    nc.tensor.matmul(
                        ps[:, :],
                        wk_sb[:, k, 128 * m : 128 * (m + 1)],
                        kt[n][:, k, :],
                        start=(k == 0),
                        stop=(k == KD - 1),
                    )
                nc.vector.tensor_copy(k_sb[:, m, CH * n : CH * (n + 1)], ps[:, :])

            def oproj_block(lb):
                # psO pool: avoids contending with scores for "sps" slots.
                # Both 512-wide d-groups evacuate into one tile -> a single
                # 256 KiB row DMA (better rate, half the SP-queue slots).
                ot = op_.tile([128, NDC, DC], bf16, tag="ot", name=f"ot{lb}")
                for dc in range(NDC):
                    pso = psO.tile([128, DC], f32, tag="ops",
                                   name=f"op{lb}_{dc}")
                    for mm in range(MB):
                        nc.tensor.matmul(
                            pso[:, :],
                            o_sb[:, mm, 128 * lb : 128 * (lb + 1)],
                            wo_sb[:, mm, DC * dc : DC * (dc + 1)],
                            start=(mm == 0),
                            stop=(mm == MB - 1),
                        )
                    nc.vector.tensor_copy(ot[:, dc, :], pso[:, :])
                nc.sync.dma_start(
                    out[128 * lb : 128 * (lb + 1), :], ot[:, :, :]
                )

            # ---- attention: one chunk, all 4 head pairs as a single
            # software-pipelined stream (scores/exp lead AV by LAG blocks,
            # continuing straight across pair boundaries so the exp backlog
            # never stalls the tensor engine). 2-block batches halve the PE
            # tiling-mode switches.
            def scores_block(st, m, i, n):
                nsb = SBC * (n + 1)
                g, ii = divmod(i, SBC)
                diag = g == n
                c0 = 128 * ii if diag else 0
                if ii == 0:
                    st[m, "p4", g] = p4p.tile(
                        [128, 2, SBC, CH], bf16, tag="p4",
                        name=f"p4_{m}_{n}_{g}")
                p4 = st[m, "p4", g]
                sps = psS.tile([128, 2, CH], f32, tag="sps",
                               name=f"sps{m}_{n}_{i}")
                nc.tensor.matmul(
                    sps[:, 0, c0:CH],
                    k_sb[0:64, m, 128 * i : 128 * (i + 1)],
                    q_sb[0:64, m, CH * n + c0 : CH * (n + 1)],
                    start=True, stop=True,
                    tile_position=(0, 0),
                )
                nc.tensor.matmul(
                    sps[:, 1, c0:CH],
                    k_sb[64:128, m, 128 * i : 128 * (i + 1)],
                    q_sb[64:128, m, CH * n + c0 : CH * (n + 1)],
                    start=True, stop=True,
                    tile_position=(64, 0),
                )
                nc.scalar.activation(
                    p4[:, :, ii, c0:CH], sps[:, :, c0:CH], Exp, scale=scale,
                )
                if diag:  # causal mask on the 128-wide diagonal window
                    nc.vector.tensor_mul(
                        p4[:, :, ii, c0 : c0 + 128],
                        p4[:, :, ii, c0 : c0 + 128],
                        tri2[:, :, :],
                    )

            def av_block(st, m, i, n):
                nsb = SBC * (n + 1)
                g, ii = divmod(i, SBC)
                diag = g == n
                c0 = 128 * ii if diag else 0
                if i == 0:
                    st[m, "ops"] = psO.tile([128, 2, CH], f32, tag="ops",
                                            name=f"ops{m}_{n}")
                ops = st[m, "ops"]
                p4 = st[m, "p4", g]
                for hh in range(2):
                    nc.tensor.matmul(
                        ops[:, hh, c0:CH],
                        v_sb[:, i, 2 * m + hh, :],
                        p4[:, hh, ii, c0:CH],
                        start=(i == 0),
                        stop=(i == nsb - 1),
                    )

            def divide_chain(st, m, n):
                # softmax divide: denominators live in PSUM partition E
                ops = st[m, "ops"]
                rs = dp.tile([128, 2, CH], f32, tag="rs", name=f"rs{m}_{n}")
                nc.vector.tensor_copy(rs[E : E + 1, :, :], ops[E : E + 1, :, :])
                dn = dp.tile([128, 2 * CH], f32, tag="dn", name=f"dn{m}_{n}")
                nc.sync.dma_start(dn[0:1, :], rs[E : E + 1, :, :])
                rr = dp.tile([128, 2, CH], f32, tag="rr", name=f"rr{m}_{n}")
                nc.gpsimd.partition_broadcast(rr[0:E, :, :], dn[0:1, :],
                                              channels=E)
                nc.vector.reciprocal_approx_fast(rr[0:E, :, :], rr[0:E, :, :])
                o_tmp = dp.tile([128, 2, CH], bf16, tag="o_tmp",
                                name=f"otmp{m}_{n}")
                nc.vector.tensor_mul(
                    o_tmp[0:E, :, :], ops[0:E, :, :], rr[0:E, :, :]
                )
                nc.sync.dma_start(
                    o_sb[0:E, m, CH * n : CH * (n + 1)], o_tmp[0:E, 0, :]
                )
                nc.sync.dma_start(
                    o_sb[E:128, m, CH * n : CH * (n + 1)], o_tmp[0:E, 1, :]
                )

            # ---- quarter 0 projections (straight-line) ----
            for sb in range(SBC):
                vproj_block(sb, 0)
            for m in range(MB):
                qproj_block(m, 0)
            for m in range(MB):
                kproj_block(m, 0)

            # ---- chunks: attention n overlapped with projections of
            #      quarter n+1 and out-projection of chunk n-1 ----
            for n in range(NQ):
                filler = []
                if n >= 1:  # out-proj of the previous chunk: deps all met
                    for lb in range(SBC * (n - 1), SBC * n):
                        filler.append((oproj_block, (lb,)))
                if n + 1 < NQ:
                    for sb in range(SBC * (n + 1), SBC * (n + 2)):
                        filler.append((vproj_block, (sb, n + 1)))
                    for m in range(MB):
                        filler.append((qproj_block, (m, n + 1)))
                    for m in range(MB):
                        filler.append((kproj_block, (m, n + 1)))

                if n + 1 < NQ:
                    _ld(vT, vt, n + 1)
                nsb = SBC * (n + 1)
                blocks = [(m, i) for m in range(MB) for i in range(nsb)]
                LAG = 2
                st = {}
                popped = 0
                av_done = 0
                for idx in range(0, len(blocks) + LAG, 2):
                    if idx < len(blocks):
                        scores_block(st, *blocks[idx], n)
                        scores_block(st, *blocks[idx + 1], n)
                    for j in (idx - LAG, idx - LAG + 1):
                        if j < 0 or j >= len(blocks):
                            continue
                        m, i = blocks[j]
                        av_block(st, m, i, n)
                        av_done += 1
                        if i == nsb - 1:  # pair finished
                            divide_chain(st, m, n)
                            if m == 0 and n + 1 < NQ:
                                _ld(qT, qt, n + 1)
                            if m == 1 and n + 1 < NQ:
                                _ld(kT, kt, n + 1)
                    # fine-grained filler pops: spread across the whole
                    # chunk so each filler's PSUM-slot wait overlaps work
                    want = (len(filler) * av_done) // len(blocks)
                    while popped < want:
                        fn, args = filler[popped]
                        fn(*args)
                        popped += 1


            # Keep the PE busy (and the HAM clock-gate at 8/8) while the
            # final pair's softmax-divide chain drains: dependency-free
            # dummy matmuls on resident data.  Without these the ~6 us stall
            # re-throttles the PE to 1.2 GHz for the whole tail.
            psw = psS.tile([128, 512], f32, tag="sps", name="warm")
            for r in range(24):
                nc.tensor.matmul(
                    psw[:, :], k_sb[:, 0, 0:128], q_sb[:, 0, 0:512],
                    start=(r == 0), stop=(r == 23),
                )

            # tail: out-projection of the last chunk; lb 12-13's PSUM
            # groups open with e-blocks 0..2 before the final divide chain
            # lands, e-block 3 closes them after; merged row DMAs.
            open4 = {}
            for lb in (12, 13):
                for dc in range(NDC):
                    g = 2 * (lb - 12) + dc
                    pool, tag = ((psO, "ops") if g < 2 else (psS, "sps"))
                    pso = pool.tile([128, DC], f32, tag=tag,
                                    name=f"opt{lb}_{dc}")
                    for mm in range(MB - 1):
                        nc.tensor.matmul(
                            pso[:, :],
                            o_sb[:, mm, 128 * lb : 128 * (lb + 1)],
                            wo_sb[:, mm, DC * dc : DC * (dc + 1)],
                            start=(mm == 0),
                            stop=False,
                        )
                    open4[lb, dc] = pso
            for lb in (12, 13):
                ot = op_.tile([128, NDC, DC], bf16, tag="ot", name=f"ott{lb}")
                for dc in range(NDC):
                    pso = open4[lb, dc]
                    nc.tensor.matmul(
                        pso[:, :],
                        o_sb[:, MB - 1, 128 * lb : 128 * (lb + 1)],
                        wo_sb[:, MB - 1, DC * dc : DC * (dc + 1)],
                        start=False,
                        stop=True,
                    )
                    nc.vector.tensor_copy(ot[:, dc, :], pso[:, :])
                nc.sync.dma_start(
                    out[128 * lb : 128 * (lb + 1), :], ot[:, :, :]
                )
            for lb in (14, 15):
                oproj_block(lb)

    nc.compile()
    return nc


def _prep_inputs(queries, keys, values, Wq, bq, Wk, Wv, Wo):
    """Build the 8 per-core input maps (host-side shard + transpose + cast)."""
    MB = EC // 128
    in_maps = []
    qT = [np.ascontiguousarray(queries[b].T.astype(_BF16)) for b in range(B)]
    kT = [np.ascontiguousarray(keys[b].T.astype(_BF16)) for b in range(B)]
    vT = [np.ascontiguousarray(values[b].T.astype(_BF16)) for b in range(B)]
    wqs, wks, wvs, wos, bqs = [], [], [], [], []
    for g in range(GROUPS):
        sl = slice(g * EC, (g + 1) * EC)
        wqs.append(np.ascontiguousarray(Wq[sl, :].T.astype(_BF16)))
        wks.append(np.ascontiguousarray(Wk[sl, :].T.astype(_BF16)))
        wvs.append(np.ascontiguousarray(Wv[sl, :].T.astype(_BF16)))
        wos.append(np.ascontiguousarray(Wo[:, sl].T.astype(_BF16)))
        bqs.append(
            np.ascontiguousarray(
                bq[sl].astype(np.float32).reshape(MB, 128).T
            )
        )
    for c in range(NCORES):
        b, g = c // GROUPS, c % GROUPS
        in_maps.append(
            {
                "qT": qT[b], "kT": kT[b], "vT": vT[b],
                "wq": wqs[g], "wk": wks[g], "wv": wvs[g],
                "wo": wos[g], "bq": bqs[g],
            }
        )
    return in_maps


_NC_CACHE = {}


def kernel(queries, keys, values, attn_mask, Wq, bq, Wk, bk, Wv, bv, Wo, bo,
           _trace=False):
    from concourse.bass_utils import run_bass_kernel_spmd

    queries = np.asarray(queries, np.float32)
    keys = np.asarray(keys, np.float32)
    values = np.asarray(values, np.float32)
    Wq, Wk, Wv, Wo = (np.asarray(a, np.float32) for a in (Wq, Wk, Wv, Wo))
    bq, bk, bv, bo = (np.asarray(a, np.float32) for a in (bq, bk, bv, bo))

    if "nc" not in _NC_CACHE:
        _NC_CACHE["nc"] = build()
    nc = _NC_CACHE["nc"]

    in_maps = _prep_inputs(queries, keys, values, Wq, bq, Wk, Wv, Wo)
    res = run_bass_kernel_spmd(
        nc, in_maps, core_ids=list(range(NCORES)), trace=_trace
    )
    _NC_CACHE["last_results"] = res

    out = np.zeros((B, L, D), np.float32)
    for c in range(NCORES):
        out[c // GROUPS] += res.results[c]["out"].astype(np.float32)
    # bv exits through the (row-sum-1) softmax as Wo @ bv; bo is direct.
    out += (Wo @ bv + bo)[None, None, :]
    return out


# revision 50
# speedup vs baseline: 1.0487x; 1.0101x over previous
"""Trainium2 Bass kernel for nn_AttentionLayer (B=4, L=S=2048, D=1024, H=16).

Sharding: 8 cores = (batch b in 0..3) x (head-group g in 0..1); each core
handles one batch and 8 heads (512 of the 1024 q/k/v/o channels).

Per-core kernel (bf16 matmuls, fp32 PSUM), fully software-pipelined in
512-wide l/s quarters so projections, attention, out-projection and DMA all
overlap:

  - q^T/k^T are kept packed: e-block m holds heads (2m, 2m+1) on partition
    halves [0:64] / [64:128].  Scores for the two heads of a pair are issued
    as 64x128 PE row-tiles (tile_position (0,0) / (64,0), contraction E=64),
    which avoids any zero-padded per-head k layout (no memsets, single
    k-proj copy).
  - exp (ScalarE) is evaluated once per s-block over BOTH heads' score
    planes ([128, 2, W] access pattern) to halve activation call overhead.
  - causal masking multiplies the 128-wide diagonal windows of the exp
    output by a lower-block-triangular bf16 tile (VectorE), only for the
    diagonal group of each chunk.
  - AV: O^T = V_ext^T @ P^T with V_ext = [V | 1 | 0-pad to 128 cols, which
    makes LDWEIGHTS FWL-eligible]; PSUM row 64 accumulates the softmax
    denominator.  Divide: PSUM row -> SBUF -> DMA shift to partition 0 ->
    GpSimd partition-broadcast -> DVE reciprocal+multiply.
  - out-projection per l-chunk (delayed one chunk) so its matmuls and the
    fp-> bf16 output DMA overlap the next chunk's attention.
  - (bk is dropped: softmax is invariant to a per-key constant shift; bv
    and bo are added on the host: out += Wo @ bv + bo exactly.)

Host: shards/transposes/casts inputs, runs the SPMD kernel on 8 cores,
sums the two head-group partial outputs per batch, adds Wo@bv + bo.
"""

import numpy as np
import ml_dtypes

B, L, S, D, H, E = 4, 2048, 2048, 1024, 16, 64
NCORES = 8
GROUPS = 2                 # head-groups (tensor-parallel dimension)
HC = H // GROUPS           # heads per core = 8
EC = HC * E                # channels per core = 512
CH = 512                   # attention l-chunk / projection quarter size

_BF16 = ml_dtypes.bfloat16


def build(debug=False):
    import concourse.bass as bass
    import concourse.mybir as mybir
    import concourse.tile as tile
    from concourse import bacc
    from concourse.masks import make_upper_triangular

    f32 = mybir.dt.float32
    bf16 = mybir.dt.bfloat16

    KD = D // 128           # k-tiles over d = 8
    MB = EC // 128          # e-blocks (= head pairs) per core = 4
    NQ = L // CH            # quarters / chunks = 4
    SBC = CH // 128         # s-blocks per chunk = 4
    NSB = S // 128          # total s-blocks = 16
    DC = 512                # out-proj d tile
    NDC = D // DC           # = 2

    nc = bacc.Bacc(None, target_bir_lowering=False, debug=debug)

    qT = nc.dram_tensor("qT", [D, L], bf16, kind="ExternalInput")
    kT = nc.dram_tensor("kT", [D, S], bf16, kind="ExternalInput")
    vT = nc.dram_tensor("vT", [D, S], bf16, kind="ExternalInput")
    wq = nc.dram_tensor("wq", [D, EC], bf16, kind="ExternalInput")
    wk = nc.dram_tensor("wk", [D, EC], bf16, kind="ExternalInput")
    wv = nc.dram_tensor("wv", [D, EC], bf16, kind="ExternalInput")
    wo = nc.dram_tensor("wo", [EC, D], bf16, kind="ExternalInput")
    bq = nc.dram_tensor("bq", [128, MB], f32, kind="ExternalInput")
    out = nc.dram_tensor("out", [L, D], bf16, kind="ExternalOutput")

    scale = 1.0 / float(np.sqrt(E))
    Exp = mybir.ActivationFunctionType.Exp

    with tile.TileContext(nc) as tc:
        with (
            tc.tile_pool(name="persist", bufs=1) as pp,
            tc.tile_pool(name="weights", bufs=1) as wp,
            tc.tile_pool(name="inputs", bufs=4) as ip,
            tc.tile_pool(name="pexp", bufs=3) as p4p,
            tc.tile_pool(name="divp", bufs=2) as dp,
            tc.tile_pool(name="oout", bufs=3) as op_,
            tc.tile_pool(name="psS", bufs=2, space="PSUM") as psS,
            tc.tile_pool(name="psO", bufs=2, space="PSUM") as psO,
        ):
            # ---- persistent SBUF tensors ----
            q_sb = pp.tile([128, MB, L], bf16, tag="q_sb")
            k_sb = pp.tile([128, MB, S], bf16, tag="k_sb")
            v_sb = pp.tile([128, NSB, HC, 128], bf16, tag="v_sb")
            o_sb = pp.tile([128, MB, L], bf16, tag="o_sb")
            tri2 = pp.tile([128, 2, 128], bf16, tag="tri2")
            bq_t = pp.tile([128, MB], f32, tag="bq_t")

            # cols E..127 of each per-head V stationary: a ones column at
            # E (softmax denominator accumulates in PSUM partition E) then
            # zero padding to 128 columns so LDWEIGHTS takes the FWL path
            # (emitted first: runs on idle DVE during the initial DMA wait)
            nc.vector.memset(v_sb[:, :, :, E + 1 :], 0.0)
            nc.vector.memset(v_sb[:, :, :, E : E + 1], 1.0)
            for hh in range(2):
                make_upper_triangular(nc, tri2[:, hh, :], val=1.0, diag=True)
            nc.sync.dma_start(bq_t[:], bq[:])

            wq_sb = wp.tile([128, KD, EC], bf16, tag="wq")
            wk_sb = wp.tile([128, KD, EC], bf16, tag="wk")
            wv_sb = wp.tile([128, KD, EC], bf16, tag="wv")
            wo_sb = wp.tile([128, MB, D], bf16, tag="wo")

            # Weights: big DMAs (341 GB/s vs ~138 at 128 KiB) on the ACT
            # HWDGE ring so they don't queue behind input slabs; first wv
            # piece lands in ~1.7 us so the first v-proj matmul starts early.
            for w_sb, w_dram in ((wv_sb, wv), (wq_sb, wq), (wk_sb, wk)):
                for s in range(2):
                    k0 = 4 * s
                    nc.scalar.dma_start(
                        w_sb[:, k0 : k0 + 4, :],
                        w_dram[128 * k0 : 128 * (k0 + 4), :].rearrange(
                            "(k p) c -> p k c", p=128
                        ),
                    )
            nc.scalar.dma_start(
                wo_sb[:], wo[:, :].rearrange("(m p) c -> p m c", p=128)
            )

            vt, qt, kt = {}, {}, {}

            def _ld(dram, d, n, nsplit=1):
                # one 1 MiB slab per (tensor, quarter): [128, KD, CH].
                # nsplit>1 lands the slab in k-tile pieces so quarter-0
                # consumers can start before the whole slab arrives.
                tq = ip.tile([128, KD, CH], bf16, tag="inq",
                             name=f"in_{dram.name}_{n}")
                kstep = KD // nsplit
                for s in range(nsplit):
                    k0 = s * kstep
                    nc.sync.dma_start(
                        tq[:, k0 : k0 + kstep, :],
                        dram[128 * k0 : 128 * (k0 + kstep),
                             CH * n : CH * (n + 1)].rearrange(
                            "(k p) c -> p k c", p=128
                        ),
                    )
                d[n] = tq

            # quarter-0 slabs, consumption order, in k-tile pieces so the
            # first projection matmuls start after ~256 KiB, not 1 MiB
            _ld(vT, vt, 0, nsplit=4)
            _ld(qT, qt, 0, nsplit=2)
            _ld(kT, kt, 0, nsplit=2)


            # ---- projection building blocks ----
            def vproj_block(sb, n):
                ps = psS.tile([128, EC], f32, tag="sps", name=f"vp{sb}")
                qq = sb % SBC
                for k in range(KD):
                    nc.tensor.matmul(
                        ps[:, :],
                        vt[n][:, k, 128 * qq : 128 * (qq + 1)],
                        wv_sb[:, k, :],
                        start=(k == 0),
                        stop=(k == KD - 1),
                    )
                nc.vector.tensor_copy(
                    v_sb[:, sb, :, 0:E],
                    ps[:, :].rearrange("p (h e) -> p h e", h=HC),
                )

            def qproj_block(m, n):
                ps = psS.tile([128, CH], f32, tag="sps", name=f"qp{m}_{n}")
                for k in range(KD):
                    nc.tensor.matmul(
                        ps[:, :],
                        wq_sb[:, k, 128 * m : 128 * (m + 1)],
                        qt[n][:, k, :],
                        start=(k == 0),
                        stop=(k == KD - 1),
                    )
                nc.vector.tensor_scalar_add(
                    q_sb[:, m, CH * n : CH * (n + 1)], ps[:, :], bq_t[:, m : m + 1]
                )

            def kproj_block(m, n):
                ps = psS.tile([128, CH], f32, tag="sps", name=f"kp{m}_{n}")
                for k in range(KD):
                    nc.tensor.matmul(
                        ps[:, :],
                        wk_sb[:, k, 128 * m : 128 * (m + 1)],
                        kt[n][:, k, :],
                        start=(k == 0),
                        stop=(k == KD - 1),
                    )
                nc.vector.tensor_copy(k_sb[:, m, CH * n : CH * (n + 1)], ps[:, :])

            def oproj_block(lb):
                # psO pool: avoids contending with scores for "sps" slots.
                # Both 512-wide d-groups evacuate into one tile -> a single
                # 256 KiB row DMA (better rate, half the SP-queue slots).
                ot = op_.tile([128, NDC, DC], bf16, tag="ot", name=f"ot{lb}")
                for dc in range(NDC):
                    pso = psO.tile([128, DC], f32, tag="ops",
                                   name=f"op{lb}_{dc}")
                    for mm in range(MB):
                        nc.tensor.matmul(
                            pso[:, :],
                            o_sb[:, mm, 128 * lb : 128 * (lb + 1)],
                            wo_sb[:, mm, DC * dc : DC * (dc + 1)],
                            start=(mm == 0),
                            stop=(mm == MB - 1),
                        )
                    nc.vector.tensor_copy(ot[:, dc, :], pso[:, :])
                nc.sync.dma_start(
                    out[128 * lb : 128 * (lb + 1), :], ot[:, :, :]
                )

            # ---- attention: one chunk, all 4 head pairs as a single
            # software-pipelined stream (scores/exp lead AV by LAG blocks,
            # continuing straight across pair boundaries so the exp backlog
            # never stalls the tensor engine). 2-block batches halve the PE
            # tiling-mode switches.
            def scores_block(st, m, i, n):
                nsb = SBC * (n + 1)
                g, ii = divmod(i, SBC)
                diag = g == n
                c0 = 128 * ii if diag else 0
                if ii == 0:
                    st[m, "p4", g] = p4p.tile(
                        [128, 2, SBC, CH], bf16, tag="p4",
                        name=f"p4_{m}_{n}_{g}")
                p4 = st[m, "p4", g]
                sps = psS.tile([128, 2, CH], f32, tag="sps",
                               name=f"sps{m}_{n}_{i}")
                nc.tensor.matmul(
                    sps[:, 0, c0:CH],
                    k_sb[0:64, m, 128 * i : 128 * (i + 1)],
                    q_sb[0:64, m, CH * n + c0 : CH * (n + 1)],
                    start=True, stop=True,
                    tile_position=(0, 0),
                )
                nc.tensor.matmul(
                    sps[:, 1, c0:CH],
                    k_sb[64:128, m, 128 * i : 128 * (i + 1)],
                    q_sb[64:128, m, CH * n + c0 : CH * (n + 1)],
                    start=True, stop=True,
                    tile_position=(64, 0),
                )
                nc.scalar.activation(
                    p4[:, :, ii, c0:CH], sps[:, :, c0:CH], Exp, scale=scale,
                )
                if diag:  # causal mask on the 128-wide diagonal window
                    nc.vector.tensor_mul(
                        p4[:, :, ii, c0 : c0 + 128],
                        p4[:, :, ii, c0 : c0 + 128],
                        tri2[:, :, :],
                    )

            def av_block(st, m, i, n):
                nsb = SBC * (n + 1)
                g, ii = divmod(i, SBC)
                diag = g == n
                c0 = 128 * ii if diag else 0
                if i == 0:
                    st[m, "ops"] = psO.tile([128, 2, CH], f32, tag="ops",
                                            name=f"ops{m}_{n}")
                ops = st[m, "ops"]
                p4 = st[m, "p4", g]
                for hh in range(2):
                    nc.tensor.matmul(
                        ops[:, hh, c0:CH],
                        v_sb[:, i, 2 * m + hh, :],
                        p4[:, hh, ii, c0:CH],
                        start=(i == 0),
                        stop=(i == nsb - 1),
                    )

            def divide_chain(st, m, n):
                # softmax divide: denominators live in PSUM partition E
                ops = st[m, "ops"]
                rs = dp.tile([128, 2, CH], f32, tag="rs", name=f"rs{m}_{n}")
                nc.vector.tensor_copy(rs[E : E + 1, :, :], ops[E : E + 1, :, :])
                dn = dp.tile([128, 2 * CH], f32, tag="dn", name=f"dn{m}_{n}")
                nc.sync.dma_start(dn[0:1, :], rs[E : E + 1, :, :])
                rr = dp.tile([128, 2, CH], f32, tag="rr", name=f"rr{m}_{n}")
                nc.gpsimd.partition_broadcast(rr[0:E, :, :], dn[0:1, :],
                                              channels=E)
                nc.vector.reciprocal_approx_fast(rr[0:E, :, :], rr[0:E, :, :])
                o_tmp = dp.tile([128, 2, CH], bf16, tag="o_tmp",
                                name=f"otmp{m}_{n}")
                nc.vector.tensor_mul(
                    o_tmp[0:E, :, :], ops[0:E, :, :], rr[0:E, :, :]
                )
                nc.sync.dma_start(
                    o_sb[0:E, m, CH * n : CH * (n + 1)], o_tmp[0:E, 0, :]
                )
                nc.sync.dma_start(
                    o_sb[E:128, m, CH * n : CH * (n + 1)], o_tmp[0:E, 1, :]
                )

            # ---- quarter 0 projections (straight-line) ----
            for sb in range(SBC):
                vproj_block(sb, 0)
            for m in range(MB):
                qproj_block(m, 0)
            for m in range(MB):
                kproj_block(m, 0)

            # ---- chunks: attention n overlapped with projections of
            #      quarter n+1 and out-projection of chunk n-1 ----
            for n in range(NQ):
                filler = []
                if n >= 1:  # out-proj of the previous chunk: deps all met
                    for lb in range(SBC * (n - 1), SBC * n):
                        filler.append((oproj_block, (lb,)))
                if n + 1 < NQ:
                    for sb in range(SBC * (n + 1), SBC * (n + 2)):
                        filler.append((vproj_block, (sb, n + 1)))
                    for m in range(MB):
                        filler.append((qproj_block, (m, n + 1)))
                    for m in range(MB):
                        filler.append((kproj_block, (m, n + 1)))

                if n + 1 < NQ:
                    _ld(vT, vt, n + 1)
                nsb = SBC * (n + 1)
                blocks = [(m, i) for m in range(MB) for i in range(nsb)]
                LAG = 2
                st = {}
                popped = 0
                av_done = 0
                for idx in range(0, len(blocks) + LAG, 2):
                    if idx < len(blocks):
                        scores_block(st, *blocks[idx], n)
                        scores_block(st, *blocks[idx + 1], n)
                    for j in (idx - LAG, idx - LAG + 1):
                        if j < 0 or j >= len(blocks):
                            continue
                        m, i = blocks[j]
                        av_block(st, m, i, n)
                        av_done += 1
                        if i == nsb - 1:  # pair finished
                            divide_chain(st, m, n)
                            if m == 0 and n + 1 < NQ:
                                _ld(qT, qt, n + 1)
                            if m == 1 and n + 1 < NQ:
                                _ld(kT, kt, n + 1)
                    # fine-grained filler pops: spread across the whole
                    # chunk so each filler's PSUM-slot wait overlaps work
                    want = (len(filler) * av_done) // len(blocks)
                    while popped < want:
                        fn, args = filler[popped]
                        fn(*args)
                        popped += 1


            # Keep the PE busy (and the HAM clock-gate at 8/8) while the
            # final pair's softmax-divide chain drains: dependency-free
            # dummy matmuls on resident data.  Without these the ~6 us stall
            # re-throttles the PE to 1.2 GHz for the whole tail.
            psw = psS.tile([128, 128], f32, tag="sps", name="warm")
            for r in range(88):
                nc.tensor.matmul(
                    psw[:, :], k_sb[:, 0, 0:128], q_sb[:, 0, 0:128],
                    start=(r == 0), stop=(r == 87),
                )

            # tail: out-projection of the last chunk; lb 12-13's PSUM
            # groups open with e-blocks 0..2 before the final divide chain
            # lands, e-block 3 closes them after; merged row DMAs.
            open4 = {}
            for lb in (12, 13):
                for dc in range(NDC):
                    g = 2 * (lb - 12) + dc
                    pool, tag = ((psO, "ops") if g < 2 else (psS, "sps"))
                    pso = pool.tile([128, DC], f32, tag=tag,
                                    name=f"opt{lb}_{dc}")
                    for mm in range(MB - 1):
                        nc.tensor.matmul(
                            pso[:, :],
                            o_sb[:, mm, 128 * lb : 128 * (lb + 1)],
                            wo_sb[:, mm, DC * dc : DC * (dc + 1)],
                            start=(mm == 0),
                            stop=False,
                        )
                    open4[lb, dc] = pso
            for lb in (12, 13):
                ot = op_.tile([128, NDC, DC], bf16, tag="ot", name=f"ott{lb}")
                for dc in range(NDC):
                    pso = open4[lb, dc]
                    nc.tensor.matmul(
                        pso[:, :],
                        o_sb[:, MB - 1, 128 * lb : 128 * (lb + 1)],
                        wo_sb[:, MB - 1, DC * dc : DC * (dc + 1)],
                        start=False,
                        stop=True,
                    )
                    nc.vector.tensor_copy(ot[:, dc, :], pso[:, :])
                nc.sync.dma_start(
                    out[128 * lb : 128 * (lb + 1), :], ot[:, :, :]
                )
            for lb in (14, 15):
                oproj_block(lb)

    nc.compile()
    return nc


def _prep_inputs(queries, keys, values, Wq, bq, Wk, Wv, Wo):
    """Build the 8 per-core input maps (host-side shard + transpose + cast)."""
    MB = EC // 128
    in_maps = []
    qT = [np.ascontiguousarray(queries[b].T.astype(_BF16)) for b in range(B)]
    kT = [np.ascontiguousarray(keys[b].T.astype(_BF16)) for b in range(B)]
    vT = [np.ascontiguousarray(values[b].T.astype(_BF16)) for b in range(B)]
    wqs, wks, wvs, wos, bqs = [], [], [], [], []
    for g in range(GROUPS):
        sl = slice(g * EC, (g + 1) * EC)
        wqs.append(np.ascontiguousarray(Wq[sl, :].T.astype(_BF16)))
        wks.append(np.ascontiguousarray(Wk[sl, :].T.astype(_BF16)))
        wvs.append(np.ascontiguousarray(Wv[sl, :].T.astype(_BF16)))
        wos.append(np.ascontiguousarray(Wo[:, sl].T.astype(_BF16)))
        bqs.append(
            np.ascontiguousarray(
                bq[sl].astype(np.float32).reshape(MB, 128).T
            )
        )
    for c in range(NCORES):
        b, g = c // GROUPS, c % GROUPS
        in_maps.append(
            {
                "qT": qT[b], "kT": kT[b], "vT": vT[b],
                "wq": wqs[g], "wk": wks[g], "wv": wvs[g],
                "wo": wos[g], "bq": bqs[g],
            }
        )
    return in_maps


_NC_CACHE = {}


def kernel(queries, keys, values, attn_mask, Wq, bq, Wk, bk, Wv, bv, Wo, bo,
           _trace=False):
    from concourse.bass_utils import run_bass_kernel_spmd

    queries = np.asarray(queries, np.float32)
    keys = np.asarray(keys, np.float32)
    values = np.asarray(values, np.float32)
    Wq, Wk, Wv, Wo = (np.asarray(a, np.float32) for a in (Wq, Wk, Wv, Wo))
    bq, bk, bv, bo = (np.asarray(a, np.float32) for a in (bq, bk, bv, bo))

    if "nc" not in _NC_CACHE:
        _NC_CACHE["nc"] = build()
    nc = _NC_CACHE["nc"]

    in_maps = _prep_inputs(queries, keys, values, Wq, bq, Wk, Wv, Wo)
    res = run_bass_kernel_spmd(
        nc, in_maps, core_ids=list(range(NCORES)), trace=_trace
    )
    _NC_CACHE["last_results"] = res

    out = np.zeros((B, L, D), np.float32)
    for c in range(NCORES):
        out[c // GROUPS] += res.results[c]["out"].astype(np.float32)
    # bv exits through the (row-sum-1) softmax as Wo @ bv; bo is direct.
    out += (Wo @ bv + bo)[None, None, :]
    return out
